# revision 1
# baseline (speedup 1.0000x reference)
"""TRN2 Bass kernel for nn_MultiHeadHyperedgeAttention.

Pipeline (8 NeuronCores, hyperedge-sharded, no collectives):
  host: sort edges by hyperedge; bin-pack segments into bins of <=64 slots
        with <=128 edges per node-shard (4 shards of 25000 rows so gather
        indices fit int16); build per-edge slot/weight tables.
  dev:  dma_gather x rows per (chunk, shard) on 4 SWDGE queues ->
        per-bin matmul G^T @ M accumulated over shards in PSUM
        (M = scaled one-hot built on-device via tensor_scalar) ->
        batched per-head MLP (3 matmuls + ACT ops) over all slots ->
        one f32 per slot.
  host: scatter slot outputs back to the [50000] output.
"""
import numpy as np

import concourse.bass as bass
import concourse.tile as tile
from concourse import bacc, mybir
from concourse.library_config import mlp as mlp_lib
from concourse.bass_utils import run_bass_kernel_spmd

NUM_NODES = 100000
NUM_HYPEREDGES = 50000
IN_DIM = 128
N_CORES = 8
N_SHARDS = 4
SHARD = NUM_NODES // N_SHARDS      # 25000 rows -> int16-safe gather indices
SLOTS = 64                         # segment slots per bin
BINCAP = 128                       # per-shard edge capacity per bin
KB = 16                            # bins per gather chunk
NIDX = KB * BINCAP                 # indices per dma_gather call
PAD_SLOT = 999.0
P = 128
D = IN_DIM
F32 = mybir.dt.float32
I16 = mybir.dt.int16
AF = mybir.ActivationFunctionType
OP = mybir.AluOpType
SIG_LO = 1.0 / (1.0 + np.exp(5.0))
SIG_HI = 1.0 / (1.0 + np.exp(-5.0))


# ---------------------------------------------------------------- host packing

def _pack(node_idx, hyperedge_idx):
    node_idx = np.asarray(node_idx, dtype=np.int64)
    hyperedge_idx = np.asarray(hyperedge_idx, dtype=np.int64)
    counts = np.bincount(hyperedge_idx, minlength=NUM_HYPEREDGES)
    inv_cnt = 1.0 / np.maximum(counts, 1).astype(np.float64)

    shard_of_edge = node_idx // SHARD
    order = np.lexsort((node_idx, shard_of_edge, hyperedge_idx))
    e_node = node_idx[order]
    e_shard = shard_of_edge[order]

    cnt_ss = np.zeros((NUM_HYPEREDGES, N_SHARDS), dtype=np.int64)
    np.add.at(cnt_ss, (hyperedge_idx, shard_of_edge), 1)
    seg_starts = np.zeros(NUM_HYPEREDGES + 1, dtype=np.int64)
    seg_starts[1:] = np.cumsum(counts)

    # segments whose per-shard edge count exceeds one bin go to the host
    # fallback path (never happens for the target distribution)
    fallback = np.where(cnt_ss.max(axis=1) > BINCAP)[0]
    fb = set(fallback.tolist())

    seg_per_core = NUM_HYPEREDGES // N_CORES
    cores = []
    for c in range(N_CORES):
        s0, s1 = c * seg_per_core, (c + 1) * seg_per_core
        bins, cur_segs = [], []
        cur_cnt = np.zeros(N_SHARDS, dtype=np.int64)
        for s in range(s0, s1):
            if s in fb:
                continue
            csm = cnt_ss[s]
            if cur_segs and (len(cur_segs) >= SLOTS or np.any(cur_cnt + csm > BINCAP)):
                bins.append((cur_segs, cur_cnt))
                cur_segs, cur_cnt = [], np.zeros(N_SHARDS, dtype=np.int64)
            cur_segs = cur_segs + [s]
            cur_cnt = cur_cnt + csm
        if cur_segs:
            bins.append((cur_segs, cur_cnt))
        cores.append(bins)

    nbins = max(len(b) for b in cores)
    nbins = -(-nbins // KB) * KB
    nchunks = nbins // KB

    idx16 = np.zeros((N_CORES, N_SHARDS, nbins, BINCAP), dtype=np.int16)
    slotf = np.full((N_CORES, nbins, BINCAP, N_SHARDS), PAD_SLOT, dtype=np.float32)
    wf = np.zeros((N_CORES, nbins, BINCAP, N_SHARDS), dtype=np.float32)
    out_map = np.full((N_CORES, nbins, SLOTS), -1, dtype=np.int64)

    for c in range(N_CORES):
        for b, (segs, _cnt) in enumerate(cores[c]):
            out_map[c, b, :len(segs)] = segs
            pos = np.zeros(N_SHARDS, dtype=np.int64)
            for sl, s in enumerate(segs):
                e0, e1 = seg_starts[s], seg_starts[s + 1]
                nodes = e_node[e0:e1]
                shards = e_shard[e0:e1]
                for sh in range(N_SHARDS):
                    msk = shards == sh
                    k = int(msk.sum())
                    if k == 0:
                        continue
                    p0 = pos[sh]
                    idx16[c, sh, b, p0:p0 + k] = (nodes[msk] - sh * SHARD).astype(np.int16)
                    slotf[c, b, p0:p0 + k, sh] = sl
                    wf[c, b, p0:p0 + k, sh] = inv_cnt[s]
                    pos[sh] += k
            # sort each shard's 128 positions by node id for HBM locality
            for sh in range(N_SHARDS):
                o = np.argsort(idx16[c, sh, b], kind="stable")
                idx16[c, sh, b] = idx16[c, sh, b][o]
                slotf[c, b, :, sh] = slotf[c, b, o, sh]
                wf[c, b, :, sh] = wf[c, b, o, sh]

    nidx = KB * BINCAP
    gidx = np.zeros((N_CORES, N_SHARDS, nchunks, P, nidx // 16), dtype=np.int16)
    for c in range(N_CORES):
        for sh in range(N_SHARDS):
            flat = idx16[c, sh].reshape(nchunks, nidx)
            # wrapped layout: idx i -> partition i%16 (tiled x8), col i//16
            w = flat.reshape(nchunks, nidx // 16, 16).transpose(0, 2, 1)
            gidx[c, sh] = np.tile(w, (1, 8, 1))

    meta = dict(nbins=nbins, nchunks=nchunks, nslots=nbins * SLOTS)
    return dict(gidx=gidx, slotf=slotf, wf=wf, out_map=out_map,
                fallback=fallback, meta=meta)


def _make_mlp_consts(W1, b1, W2, b2):
    W1 = np.asarray(W1, np.float32); b1 = np.asarray(b1, np.float32)
    W2 = np.asarray(W2, np.float32); b2 = np.asarray(b2, np.float32)
    H, Din, K = W1.shape
    w1cat = np.ascontiguousarray(W1.transpose(1, 0, 2).reshape(Din, H * K))
    w2blk = np.zeros((H * K, H), np.float32)
    for h in range(H):
        w2blk[h * K:(h + 1) * K, h] = W2[h]
    return dict(w1cat=w1cat, b1cat=b1.reshape(H * K, 1),
                w2blk=w2blk, b2col=b2.reshape(H, 1),
                meanw=np.full((H, 1), 0.9 / H, np.float32))


def _make_in_map(core, x, packed, consts):
    m = packed["meta"]
    nchunks = m["nchunks"]
    slotf, wf = packed["slotf"][core], packed["wf"][core]
    sl = slotf.reshape(nchunks, KB, P, N_SHARDS).transpose(0, 2, 1, 3)
    ww = wf.reshape(nchunks, KB, P, N_SHARDS).transpose(0, 2, 1, 3)
    slotw = np.empty((nchunks, P, KB * 8), np.float32)
    slotw.reshape(nchunks, P, KB, 8)[:, :, :, 0:4] = sl
    slotw.reshape(nchunks, P, KB, 8)[:, :, :, 4:8] = ww
    im = {
        "gidx": packed["gidx"][core],
        "slotw": slotw,
        "iota64": np.tile(np.arange(SLOTS, dtype=np.float32)[None, :], (P, 1)),
        **consts,
    }
    for s in range(N_SHARDS):
        im[f"xs{s}"] = np.ascontiguousarray(x[s * SHARD:(s + 1) * SHARD])
    return im


# ---------------------------------------------------------------- device kernel

def build_nc(nbins, nchunks, n_cores, mlp_chunk=512, repeat=1):
    nslots = nbins * SLOTS
    assert nchunks * KB == nbins and nslots % mlp_chunk == 0
    nc = bacc.Bacc("TRN2", target_bir_lowering=False, debug=False,
                   num_devices=n_cores, num_swdge_queues=4)
    xs = [nc.dram_tensor(f"xs{s}", [SHARD, D], F32, kind="ExternalInput").ap()
          for s in range(N_SHARDS)]
    gidx = nc.dram_tensor("gidx", [N_SHARDS, nchunks, P, NIDX // 16], I16,
                          kind="ExternalInput").ap()
    slotw = nc.dram_tensor("slotw", [nchunks, P, KB * 8], F32,
                           kind="ExternalInput").ap()
    iota_d = nc.dram_tensor("iota64", [P, SLOTS], F32, kind="ExternalInput").ap()
    w1_d = nc.dram_tensor("w1cat", [D, 64], F32, kind="ExternalInput").ap()
    b1_d = nc.dram_tensor("b1cat", [64, 1], F32, kind="ExternalInput").ap()
    w2_d = nc.dram_tensor("w2blk", [64, 8], F32, kind="ExternalInput").ap()
    b2_d = nc.dram_tensor("b2col", [8, 1], F32, kind="ExternalInput").ap()
    mean_d = nc.dram_tensor("meanw", [8, 1], F32, kind="ExternalInput").ap()
    out_d = nc.dram_tensor("out", [1, nslots], F32, kind="ExternalOutput").ap()

    with tile.TileContext(nc) as tc:
        with (
            tc.tile_pool(name="consts", bufs=1) as cpool,
            tc.tile_pool(name="idx", bufs=8) as ipool,
            tc.tile_pool(name="g", bufs=8) as gpool,
            tc.tile_pool(name="slotwp", bufs=3) as spool,
            tc.tile_pool(name="m4w", bufs=3) as mpool,
            tc.tile_pool(name="feats", bufs=1) as fpool,
            tc.tile_pool(name="mlptmp", bufs=3) as tpool,
            tc.tile_pool(name="outp", bufs=1) as opool,
            tc.tile_pool(name="psf", bufs=4, space="PSUM") as psf,
            tc.tile_pool(name="psh", bufs=2, space="PSUM") as psh,
            tc.tile_pool(name="psa", bufs=1, space="PSUM") as psa,
            tc.tile_pool(name="pso", bufs=1, space="PSUM") as pso,
        ):
            nc.gpsimd.load_library(mlp_lib)
            iota_t = cpool.tile([P, SLOTS], F32)
            nc.sync.dma_start(out=iota_t[:], in_=iota_d[:])
            w1_t = cpool.tile([D, 64], F32)
            nc.sync.dma_start(out=w1_t[:], in_=w1_d[:])
            b1_t = cpool.tile([64, 1], F32)
            nc.sync.dma_start(out=b1_t[:], in_=b1_d[:])
            w2_t = cpool.tile([64, 8], F32)
            nc.sync.dma_start(out=w2_t[:], in_=w2_d[:])
            b2_t = cpool.tile([8, 1], F32)
            nc.sync.dma_start(out=b2_t[:], in_=b2_d[:])
            mean_t = cpool.tile([8, 1], F32)
            nc.sync.dma_start(out=mean_t[:], in_=mean_d[:])

            featsT = fpool.tile([P, nslots], F32)
            out_sb = opool.tile([1, nslots], F32)

            for _r in range(repeat):
                for ch in range(nchunks):
                    gts = []
                    for s in range(N_SHARDS):
                        it = ipool.tile([P, NIDX // 16], I16, tag="idx")
                        nc.sync.dma_start(out=it[:], in_=gidx[s, ch])
                        G = gpool.tile([P, KB, D], F32, tag="G")
                        nc.gpsimd.dma_gather(G[:], xs[s][:], it[:], NIDX, NIDX, D,
                                             single_packet=False, queue_num=s)
                        gts.append(G)
                    sw = spool.tile([P, KB * 8], F32, tag="sw")
                    nc.sync.dma_start(out=sw[:], in_=slotw[ch])
                    for k in range(KB):
                        b = ch * KB + k
                        m4 = mpool.tile([P, N_SHARDS * SLOTS], F32, tag="m4")
                        for s in range(N_SHARDS):
                            nc.vector.tensor_scalar(
                                out=m4[:, s * SLOTS:(s + 1) * SLOTS],
                                in0=iota_t[:],
                                scalar1=sw[:, k * 8 + s:k * 8 + s + 1],
                                scalar2=sw[:, k * 8 + 4 + s:k * 8 + 4 + s + 1],
                                op0=OP.is_equal, op1=OP.mult)
                        pf = psf.tile([P, SLOTS], F32, tag="pf")
                        for s in range(N_SHARDS):
                            nc.tensor.matmul(
                                out=pf[:], lhsT=gts[s][:, k, :],
                                rhs=m4[:, s * SLOTS:(s + 1) * SLOTS],
                                start=(s == 0), stop=(s == N_SHARDS - 1))
                        nc.scalar.copy(out=featsT[:, b * SLOTS:(b + 1) * SLOTS],
                                       in_=pf[:])

                for j in range(nslots // mlp_chunk):
                    cols = slice(j * mlp_chunk, (j + 1) * mlp_chunk)
                    ph = psh.tile([64, mlp_chunk], F32, tag="ph")
                    nc.tensor.matmul(out=ph[:], lhsT=w1_t[:], rhs=featsT[:, cols],
                                     start=True, stop=True)
                    hr = tpool.tile([64, mlp_chunk], F32, tag="hr")
                    nc.scalar.activation(out=hr[:], in_=ph[:], func=AF.Relu,
                                         bias=b1_t[:])
                    pa = psa.tile([8, mlp_chunk], F32, tag="pa")
                    nc.tensor.matmul(out=pa[:], lhsT=w2_t[:], rhs=hr[:],
                                     start=True, stop=True)
                    sg = tpool.tile([8, mlp_chunk], F32, tag="sg")
                    nc.scalar.activation(out=sg[:], in_=pa[:], func=AF.Sigmoid,
                                         bias=b2_t[:])
                    nc.vector.tensor_scalar(out=sg[:], in0=sg[:],
                                            scalar1=float(SIG_LO),
                                            scalar2=float(SIG_HI),
                                            op0=OP.max, op1=OP.min)
                    po = pso.tile([1, mlp_chunk], F32, tag="po")
                    nc.tensor.matmul(out=po[:], lhsT=mean_t[:], rhs=sg[:],
                                     start=True, stop=True)
                    nc.scalar.activation(out=out_sb[:, cols], in_=po[:],
                                         func=AF.Copy, bias=0.1)
            nc.sync.dma_start(out=out_d[:], in_=out_sb[:])
    nc.compile()
    return nc


# ---------------------------------------------------------------- entry point

def _host_fallback(out, segs, x, node_idx, hyperedge_idx, W1, b1, W2, b2):
    for s in segs:
        rows = x[node_idx[hyperedge_idx == s]]
        feats = rows.mean(axis=0) if len(rows) else np.zeros(IN_DIM, np.float32)
        h = np.maximum(np.einsum("d,hdk->hk", feats, W1) + b1, 0.0)
        alpha = np.einsum("hk,hk->h", h, W2) + b2
        w = 1.0 / (1.0 + np.exp(-np.clip(alpha, -5, 5)))
        out[s] = w.mean() * 0.9 + 0.1


def kernel(x, node_idx, hyperedge_idx, W1, b1, W2, b2):
    x = np.asarray(x, np.float32)
    node_idx = np.asarray(node_idx)
    hyperedge_idx = np.asarray(hyperedge_idx)
    W1 = np.asarray(W1, np.float32); b1 = np.asarray(b1, np.float32)
    W2 = np.asarray(W2, np.float32); b2 = np.asarray(b2, np.float32)

    packed = _pack(node_idx, hyperedge_idx)
    m = packed["meta"]
    consts = _make_mlp_consts(W1, b1, W2, b2)
    nc = build_nc(m["nbins"], m["nchunks"], N_CORES)
    in_maps = [_make_in_map(c, x, packed, consts) for c in range(N_CORES)]
    res = run_bass_kernel_spmd(nc, in_maps, list(range(N_CORES)))

    out = np.full(NUM_HYPEREDGES, np.nan, dtype=np.float32)
    om = packed["out_map"].reshape(N_CORES, -1)
    for c in range(N_CORES):
        core_out = res.results[c]["out"].reshape(-1)
        v = om[c] >= 0
        out[om[c][v]] = core_out[v]
    if len(packed["fallback"]):
        _host_fallback(out, packed["fallback"], x, node_idx, hyperedge_idx,
                       W1, b1, W2, b2)
    assert not np.isnan(out).any()
    return out
